# revision 16
# baseline (speedup 1.0000x reference)
"""Mamba block on 8 trn2 NeuronCores.

Sharding: data-parallel over batch (2 groups of 4 cores) x tensor-parallel
over d_inner (4-way, 512 channels/core). Device layout is [d_channel, time]
so the selective scan runs as `tensor_tensor_scan` along the free (time)
axis. f16 AllReduce of x_dbl per group; out_proj partials summed host-side.

Time is processed in two 1024-column halves with the scans carry-chained
across the boundary. Half-1's projections/conv/AllReduce are emitted
interleaved into half-0's scan loop (and half-0's gate/out_proj into
half-1's scan loop) so the PE/ACT/DMA/collective work hides under the
DVE-bound scan phase. A tiny warmup AllReduce absorbs the first
collective's cross-core rendezvous cost.

Engine split: DVE runs the scans and the per-state tensor ops (GpSimd
shares SBUF ports with DVE, so offloading there slows both). ACT runs
native Silu, softplus as Ln(1+Exp(.)), the per-state Exp, and all
PSUM->SBUF staging copies (Copy lives in every act table set). PE runs the
projections and the B/C one-hot broadcasts.
"""

import numpy as np

from concourse import bass, mybir, tile
from concourse import bacc
from concourse.bass_utils import run_bass_kernel_spmd

# Model dims (hardcoded; harness runs kernel.py standalone)
D_MODEL = 1024
D_STATE = 16
D_CONV = 4
D_INNER = 2048
DT_RANK = 64
B_SZ, T_LEN = 2, 2048

TP = 4                      # tensor-parallel width within a batch group
DSH = D_INNER // TP         # 512 channels per core
NT = DSH // 128             # 4 d-tiles of 128 channels
CH = 512
TH = T_LEN // 2             # half length (1024)
NH = TH // CH               # 512-chunks per half (2)

F32 = mybir.dt.float32
F16 = mybir.dt.float16

MUL = mybir.AluOpType.mult
ADD = mybir.AluOpType.add
AF = mybir.ActivationFunctionType
GRPS = [[0, 1, 2, 3], [4, 5, 6, 7]]


def build_graph():
    nc = bacc.Bacc("TRN2", target_bir_lowering=False, num_devices=8)

    hsT = nc.dram_tensor("hsT", [D_MODEL, T_LEN], F16, kind="ExternalInput")
    w_inT = nc.dram_tensor("w_inT", [D_MODEL, 2 * DSH], F16, kind="ExternalInput")
    w_xT = nc.dram_tensor("w_xT", [DSH, DT_RANK + 2 * D_STATE], F16, kind="ExternalInput")
    w_dtT = nc.dram_tensor("w_dtT", [DT_RANK, DSH], F16, kind="ExternalInput")
    w_outT = nc.dram_tensor("w_outT", [DSH, D_MODEL], F16, kind="ExternalInput")
    conv_w = nc.dram_tensor("conv_w", [NT, 128, D_CONV], F32, kind="ExternalInput")
    # vecs columns: 0=conv_b, 1=b_dt, 2=D, 3=-conv_b (3 unused)
    vecs = nc.dram_tensor("vecs", [NT, 128, 4], F32, kind="ExternalInput")
    a_log = nc.dram_tensor("a_log", [NT, 128, D_STATE], F32, kind="ExternalInput")
    onehot = nc.dram_tensor("onehot", [32, 32 * 128], F16, kind="ExternalInput")
    out_d = nc.dram_tensor("out", [D_MODEL, T_LEN], F16, kind="ExternalOutput")

    with tile.TileContext(nc) as tc:
        with (
            tc.tile_pool(name="wconst", bufs=1) as wconst,
            tc.tile_pool(name="acts", bufs=1) as acts,
            tc.tile_pool(name="hsp", bufs=1) as hsp,
            tc.tile_pool(name="convp", bufs=1) as convp,
            tc.tile_pool(name="spp", bufs=2) as spp,
            tc.tile_pool(name="bcast", bufs=2) as bcast,
            tc.tile_pool(name="scantmp", bufs=2) as scantmp,
            tc.tile_pool(name="outstg", bufs=2) as outstg,
            tc.tile_pool(name="psA", bufs=2, space="PSUM") as psA,
            tc.tile_pool(name="psB", bufs=2, space="PSUM") as psB,
            tc.tile_pool(name="psO", bufs=2, space="PSUM") as psO,
            tc.tile_pool(name="dram", bufs=1, space="DRAM") as dram,
        ):
            # warmup collective first: absorbs first-CC rendezvous cost off
            # the critical path (its tiny DMA must not queue behind weights)
            wu_in = dram.tile([32, 4], F32)
            wu_out = dram.tile([32, 4], F32)
            with tc.high_priority():
                nc.sync.dma_start(wu_in[:], a_log[0, 0:32, 0:4])
                nc.gpsimd.collective_compute(
                    "AllReduce", ADD, replica_groups=GRPS,
                    ins=[wu_in.opt()], outs=[wu_out.opt()])

            # ---- resident weights/consts (w_in first: in_proj needs it) ----
            w_in_sb = wconst.tile([128, 8 * 1024], F16)
            for k in range(8):
                nc.sync.dma_start(
                    w_in_sb[:, k * 1024:(k + 1) * 1024],
                    w_inT[k * 128:(k + 1) * 128, :])
            w_x_sb = wconst.tile([128, NT * 96], F16)
            for k in range(NT):
                nc.sync.dma_start(
                    w_x_sb[:, k * 96:(k + 1) * 96],
                    w_xT[k * 128:(k + 1) * 128, :])
            w_dt_sb = wconst.tile([DT_RANK, DSH], F16)
            nc.sync.dma_start(w_dt_sb[:], w_dtT[:])
            w_out_sb = wconst.tile([128, NT * D_MODEL], F16)
            for k in range(NT):
                nc.sync.dma_start(
                    w_out_sb[:, k * D_MODEL:(k + 1) * D_MODEL],
                    w_outT[k * 128:(k + 1) * 128, :])
            conv_w_sb = wconst.tile([128, NT * D_CONV], F32)
            vecs_sb = wconst.tile([128, NT * 4], F32)
            a_log_sb = wconst.tile([128, NT * D_STATE], F32)
            for m in range(NT):
                nc.sync.dma_start(conv_w_sb[:, m * 4:(m + 1) * 4], conv_w[m])
                nc.sync.dma_start(vecs_sb[:, m * 4:(m + 1) * 4], vecs[m])
                nc.sync.dma_start(a_log_sb[:, m * 16:(m + 1) * 16], a_log[m])
            onehot_sb = wconst.tile([32, 32 * 128], F16)
            nc.sync.dma_start(onehot_sb[:], onehot[:])

            # A = -exp(A_log)
            a_sb = wconst.tile([128, NT * D_STATE], F32)
            nc.scalar.activation(a_sb[:], a_log_sb[:], AF.Exp)
            nc.vector.tensor_scalar_mul(a_sb[:], a_sb[:], -1.0)

            # ---- persistent activations ----
            x_all = [acts.tile([128, T_LEN + 3], F16, name=f"x_all{m}") for m in range(NT)]
            z_all = [acts.tile([128, T_LEN], F16, name=f"z_all{m}") for m in range(NT)]
            xc = [acts.tile([128, T_LEN], F16, name=f"xc{m}") for m in range(NT)]
            dt_sb = [acts.tile([128, T_LEN], F16, name=f"dt{m}") for m in range(NT)]
            u_sb = [acts.tile([128, T_LEN], F16, name=f"u{m}") for m in range(NT)]
            y_sb = [acts.tile([128, T_LEN], F16, name=f"y{m}") for m in range(NT)]
            carry = [acts.tile([128, D_STATE], F32, name=f"carry{m}") for m in range(NT)]
            xdbl_f16 = acts.tile([96, T_LEN], F16)
            bc_f16 = acts.tile([32, T_LEN], F16)
            for m in range(NT):
                nc.vector.memset(x_all[m][:, 0:3], 0.0)

            # ---------------- emission helpers (per half h) ----------------
            def load_hs(h):
                hs_h = hsp.tile([128, 8 * TH], F16, tag="hs")
                for k in range(8):
                    nc.sync.dma_start(
                        hs_h[:, k * TH:(k + 1) * TH],
                        hsT[k * 128:(k + 1) * 128, h * TH:(h + 1) * TH])
                return hs_h

            def in_proj_mtile(h, m, hs_h):
                """One 128-row output tile of in_proj for half h, plus its
                dependent conv+silu (x tiles) or silu (z tiles)."""
                for n in range(NH):
                    ps = psA.tile([128, CH], F32, tag="ps")
                    for k in range(8):
                        nc.tensor.matmul(
                            ps[:], w_in_sb[:, k * 1024 + m * 128: k * 1024 + (m + 1) * 128],
                            hs_h[:, k * TH + n * CH: k * TH + (n + 1) * CH],
                            start=(k == 0), stop=(k == 7))
                    if m < NT:
                        nc.scalar.activation(
                            x_all[m][:, 3 + h * TH + n * CH: 3 + h * TH + (n + 1) * CH],
                            ps[:], AF.Copy)
                    else:
                        nc.scalar.activation(
                            z_all[m - NT][:, h * TH + n * CH: h * TH + (n + 1) * CH],
                            ps[:], AF.Copy)
                if m >= NT:
                    zz = z_all[m - NT][:, h * TH:(h + 1) * TH]
                    nc.scalar.activation(zz, zz, AF.Silu)
                    return
                # conv for this x tile's half (halo cols come from h-1 / zeros)
                ta = convp.tile([128, TH], F16, tag="ta")
                tb = convp.tile([128, TH], F16, tag="tb")
                base = h * TH
                nc.vector.tensor_scalar_mul(
                    ta[:], x_all[m][:, base:base + TH], conv_w_sb[:, m * 4: m * 4 + 1])
                nc.vector.tensor_scalar_mul(
                    tb[:], x_all[m][:, base + 1:base + 1 + TH], conv_w_sb[:, m * 4 + 1: m * 4 + 2])
                nc.vector.tensor_tensor(ta[:], ta[:], tb[:], op=ADD)
                nc.vector.tensor_scalar_mul(
                    tb[:], x_all[m][:, base + 2:base + 2 + TH], conv_w_sb[:, m * 4 + 2: m * 4 + 3])
                nc.vector.tensor_tensor(ta[:], ta[:], tb[:], op=ADD)
                nc.vector.tensor_scalar_mul(
                    tb[:], x_all[m][:, base + 3:base + 3 + TH], conv_w_sb[:, m * 4 + 3: m * 4 + 4])
                nc.vector.tensor_tensor(ta[:], ta[:], tb[:], op=ADD)
                nc.scalar.activation(
                    xc[m][:, base:base + TH], ta[:], AF.Silu,
                    bias=vecs_sb[:, m * 4: m * 4 + 1])

            def xproj_ar(h):
                for n in range(NH):
                    psx = psA.tile([128, CH], F32, tag="ps")
                    off = h * TH + n * CH
                    for k in range(NT):
                        nc.tensor.matmul(
                            psx[0:96, :], w_x_sb[:, k * 96:(k + 1) * 96],
                            xc[k][:, off:off + CH],
                            start=(k == 0), stop=(k == NT - 1))
                    nc.scalar.activation(
                        xdbl_f16[:, off:off + CH], psx[0:96, :], AF.Copy)
                cc_in = dram.tile([96, TH], F16, name=f"cci{h}")
                cc_out = dram.tile([96, TH], F16, name=f"cco{h}")
                sl = slice(h * TH, (h + 1) * TH)
                nc.sync.dma_start(cc_in[:], xdbl_f16[:, sl])
                nc.gpsimd.collective_compute(
                    "AllReduce", ADD, replica_groups=GRPS,
                    ins=[cc_in.opt()], outs=[cc_out.opt()])
                nc.sync.dma_start(bc_f16[:, sl], cc_out[64:96, :])
                nc.sync.dma_start(xdbl_f16[0:64, sl], cc_out[0:64, :])

            def dt_u_y(h):
                sl = slice(h * TH, (h + 1) * TH)
                for m in range(NT):
                    spt = spp.tile([128, TH], F16, tag="spt")
                    for n in range(NH):
                        psd = psA.tile([128, CH], F32, tag="ps")
                        nc.tensor.matmul(
                            psd[:], w_dt_sb[:, m * 128:(m + 1) * 128],
                            xdbl_f16[0:DT_RANK, h * TH + n * CH: h * TH + (n + 1) * CH],
                            start=True, stop=True)
                        nc.scalar.activation(
                            spt[:, n * CH:(n + 1) * CH], psd[:], AF.Exp,
                            bias=vecs_sb[:, m * 4 + 1: m * 4 + 2])
                    nc.scalar.activation(dt_sb[m][:, sl], spt[:], AF.Ln, bias=1.0)
                    nc.vector.tensor_tensor(
                        u_sb[m][:, sl], dt_sb[m][:, sl], xc[m][:, sl], op=MUL)
                    nc.vector.tensor_scalar_mul(
                        y_sb[m][:, sl], xc[m][:, sl], vecs_sb[:, m * 4 + 2: m * 4 + 3])

            def gate(h):
                sl = slice(h * TH, (h + 1) * TH)
                for m in range(NT):
                    nc.vector.tensor_tensor(
                        y_sb[m][:, sl], y_sb[m][:, sl], z_all[m][:, sl], op=MUL)

            def out_proj(h, mo):
                for n in range(NH):
                    pso = psO.tile([128, CH], F32, tag="pso")
                    off = h * TH + n * CH
                    for k in range(NT):
                        nc.tensor.matmul(
                            pso[:],
                            w_out_sb[:, k * D_MODEL + mo * 128: k * D_MODEL + (mo + 1) * 128],
                            y_sb[k][:, off:off + CH],
                            start=(k == 0), stop=(k == NT - 1))
                    ot = outstg.tile([128, CH], F16, tag="ot")
                    nc.scalar.activation(ot[:], pso[:], AF.Copy)
                    nc.sync.dma_start(
                        out_d[mo * 128:(mo + 1) * 128, off:off + CH], ot[:])

            def scan_state(h, s):
                Bb = bcast.tile([128, TH], F16, tag="Bb")
                Cb = bcast.tile([128, TH], F16, tag="Cb")
                for src_row, dst in ((s, Bb), (16 + s, Cb)):
                    psb = psB.tile([128, TH], F32, tag="psb")
                    for n in range(NH):
                        nc.tensor.matmul(
                            psb[:, n * CH:(n + 1) * CH],
                            onehot_sb[:, src_row * 128:(src_row + 1) * 128],
                            bc_f16[:, h * TH + n * CH: h * TH + (n + 1) * CH],
                            start=True, stop=True)
                    nc.scalar.activation(dst[:], psb[:], AF.Copy)
                sl = slice(h * TH, (h + 1) * TH)
                for m in range(NT):
                    dA = scantmp.tile([128, TH], F32, tag="dA")
                    nc.scalar.activation(
                        dA[:], dt_sb[m][:, sl], AF.Exp,
                        scale=a_sb[:, m * 16 + s: m * 16 + s + 1])
                    dBx = scantmp.tile([128, TH], F16, tag="dBx")
                    nc.vector.tensor_tensor(dBx[:], u_sb[m][:, sl], Bb[:], op=MUL)
                    ht = scantmp.tile([128, TH], F32, tag="ht")
                    nc.vector.tensor_tensor_scan(
                        ht[:], dA[:], dBx[:],
                        initial=(0.0 if h == 0 else carry[m][:, s:s + 1]),
                        op0=MUL, op1=ADD)
                    if h == 0:
                        nc.scalar.activation(
                            carry[m][:, s:s + 1], ht[:, TH - 1:TH], AF.Copy)
                    # hc reuses the dA buffer (dA is dead after the scan)
                    nc.vector.tensor_tensor(dA[:], ht[:], Cb[:], op=MUL)
                    nc.vector.tensor_tensor(
                        y_sb[m][:, sl], y_sb[m][:, sl], dA[:], op=ADD)

            # ---------------- emission schedule ----------------
            # pre-work half 0 (critical path): x tiles -> xproj/AR, z tiles
            # and their silu fill the AllReduce window
            hs0 = load_hs(0)
            for m in range(NT):
                in_proj_mtile(0, m, hs0)
            xproj_ar(0)
            for m in range(NT, 8):
                in_proj_mtile(0, m, hs0)
            dt_u_y(0)
            hs1 = [None]

            # scan half 0, with half-1 pre-work interleaved
            for s in range(D_STATE):
                scan_state(0, s)
                if s in (1, 2, 3, 4):
                    if s == 1:
                        hs1[0] = load_hs(1)
                    in_proj_mtile(1, 2 * (s - 1), hs1[0])
                    in_proj_mtile(1, 2 * (s - 1) + 1, hs1[0])
                elif s == 6:
                    xproj_ar(1)
                elif s == 8:
                    dt_u_y(1)

            # scan half 1, with half-0 gate/out_proj interleaved
            for s in range(D_STATE):
                scan_state(1, s)
                if s == 1:
                    gate(0)
                elif s in (3, 5, 7, 9):
                    out_proj(0, s - 3)
                    out_proj(0, s - 2)

            gate(1)
            for mo in range(8):
                out_proj(1, mo)

    nc.finalize()
    return nc


def _onehot():
    oh = np.zeros((32, 32 * 128), np.float16)
    for s in range(32):
        oh[s, s * 128:(s + 1) * 128] = 1.0
    return oh


def make_in_maps(hidden_states, W_in, conv_w, conv_b, W_x, W_dt, b_dt, A_log, D, W_out):
    f16 = np.float16
    in_maps = []
    for core in range(8):
        g, r = divmod(core, TP)
        sh = slice(r * DSH, (r + 1) * DSH)
        m = {
            "hsT": np.ascontiguousarray(hidden_states[g].T).astype(f16),
            "w_inT": np.ascontiguousarray(
                np.concatenate([W_in[sh], W_in[D_INNER + r * DSH: D_INNER + (r + 1) * DSH]], 0).T).astype(f16),
            "w_xT": np.ascontiguousarray(W_x[:, sh].T).astype(f16),
            "w_dtT": np.ascontiguousarray(W_dt[sh].T).astype(f16),
            "w_outT": np.ascontiguousarray(W_out[:, sh].T).astype(f16),
            "conv_w": np.ascontiguousarray(conv_w[sh, 0, :]).reshape(NT, 128, D_CONV).astype(np.float32),
            "vecs": np.stack([conv_b[sh], b_dt[sh], D[sh], -conv_b[sh]], -1).reshape(NT, 128, 4).astype(np.float32),
            "a_log": np.ascontiguousarray(A_log[sh]).reshape(NT, 128, D_STATE).astype(np.float32),
            "onehot": _onehot(),
        }
        in_maps.append(m)
    return in_maps


_NC_CACHE = {}


def kernel(**inputs):
    inputs = {k: np.asarray(v) for k, v in inputs.items()}
    if "nc" not in _NC_CACHE:
        _NC_CACHE["nc"] = build_graph()
    nc = _NC_CACHE["nc"]
    in_maps = make_in_maps(**inputs)
    res = run_bass_kernel_spmd(nc, in_maps, core_ids=list(range(8)))
    outs = res.results
    full = np.zeros((B_SZ, T_LEN, D_MODEL), np.float32)
    for g in range(B_SZ):
        acc = np.zeros((D_MODEL, T_LEN), np.float32)
        for r in range(TP):
            acc += np.asarray(outs[g * TP + r]["out"], np.float32)
        full[g] = acc.T
    return full


if __name__ == "__main__":
    import reference
    ins = reference.setup_inputs()
    ins = {k: np.asarray(v) for k, v in ins.items()}
    exp = np.asarray(reference.reference(**ins))
    got = kernel(**ins)
    err = np.abs(got - exp).max() / (np.abs(exp).max() + 1e-9)
    print("Relative error:", err)
